# Initial kernel scaffold
#
"""Trainium2 Bass kernel for a dense transformer block.

Problem: B=4, N=1024, C=1024, H=16 heads (HD=64), MLP hidden 4096, pre-norm,
RoPE on q/k, exact gelu.

Sharding (8 cores, no collectives): core c handles batch b=c//2 and
sequence-half h=c%2. Each core computes LN1 + K/V over its batch's full 1024
tokens (cheap duplication), and Q / attention / proj / MLP only for its 512
local tokens. Tokens are permuted per-core so the local half is always
columns 0:512 -> all cores run an identical program.

On-chip layout is feature-major (transposed): activations live as [C_part,
token_free]; weights are stationary matmul operands (lhsT), activations
stream as the moving operand. All moving operands are bf16 (fp32(r) streams
at half the PE rate). The host pre-transposes x, pre-tiles all weights into
[out_tile][128, kchunks*128] blocks, and pre-permutes w_q/w_k columns so each
32-partition quadrant holds [re(16)|im(16)] of one head's pair-block.

RoPE: out = in*cosR + qswap(in*sinPM), where sinPM carries the +/- sign per
16-row block and qswap = ONE DVE stream_shuffle (the 16-wide re/im interleave
makes the swap a within-quadrant permute, mask [16..31,0..15]). The PSUM
result is cast to bf16 on ScalarE first; muls are bf16 on DVE.

Attention per head-pair tile j: the two heads' score matmuls (K=64) go to
row-groups {0,1}/{2,3} of the PE array and run concurrently into one
[128,2,512] PSUM pair-tile; one Exp on ScalarE covers both heads; MM2 with
lhsT=[v | ones64] (M=128) yields o_unnorm on partitions 0:64 and the softmax
denominator replicated 64x on 64:128; a DVE copy moves Z to partitions
0:64 (DVE reciprocal must not read PSUM) and vector.reciprocal + one
multiply normalize.

LayerNorm (feature-major): column sums via all-ones [128,128] bf16 stationary
matmuls accumulated over chunks (streams a bf16 cast of the input);
rstd = Exp(-0.5*Ln(var+eps)) on ScalarE -- Ln/Exp share one activation table
set with attention's Exp, so the whole kernel does only two table loads
(natural_log_exp + gelu).

NOTE: toolchain constraints this kernel respects:
- walrus allows only 1 semaphore wait per instruction (excess waits are
  split onto EventSemaphore carriers by a BIR post-pass below)
- accumulating matmuls (start=False) require K=128
- tensor_tensor operands must share the start partition; single-input ops
  (copy/activation) may cross partitions
- scalar.activation Reciprocal/Rsqrt are blocked in bass (accuracy), and
  the custom-DVE reciprocal_approx ops fail walrus codegen in this
  toolchain; LN rstd uses the Ln+Exp trick, softmax uses vector.reciprocal
"""

import json
import ml_dtypes
import numpy as np
from contextlib import ExitStack

import concourse.bass as bass
import concourse.tile as tile
from concourse import mybir
from concourse.bass_utils import run_bass_kernel_spmd

_MAXW = 1


def _split_multiwait(bir_bytes):
    """Move excess per-instruction semaphore waits onto same-engine
    EventSemaphore carriers inserted before the instruction (engine queues
    are in-order, so this is semantically identical)."""
    bir = json.loads(bir_bytes)
    n = [0]
    for fn in bir.get("functions", []):
        for bb in fn.get("blocks", []):
            out = []
            for inst in bb.get("instructions", []):
                si = inst.get("sync_info")
                ow = (si or {}).get("on_wait") or []
                if len(ow) > _MAXW:
                    excess, keep = ow[:-_MAXW], ow[-_MAXW:]
                    for s in range(0, len(excess), _MAXW):
                        n[0] += 1
                        out.append({
                            "debug": inst.get("debug", 0),
                            "engine": inst["engine"],
                            "ins": [],
                            "name": f"antsplitw-{n[0]}",
                            "opcode": "EventSemaphore",
                            "outs": [],
                            "sync_info": {"on_update": [],
                                          "on_wait": excess[s:s + _MAXW]},
                        })
                    si["on_wait"] = keep
                out.append(inst)
            bb["instructions"] = out
    return json.dumps(bir).encode()


def _install_multiwait_hook():
    import concourse.bass2jax as bass2jax
    from concourse import bass_utils as bu
    if getattr(bass2jax, "_ant_multiwait_hooked", False):
        return
    orig = bu.compile_bir_kernel

    def wrapper(bir_json, tmpdir, neff_name="file.neff"):
        if isinstance(bir_json, str):
            bir_json = bir_json.encode()
        return orig(_split_multiwait(bir_json), tmpdir, neff_name)

    bass2jax.compile_bir_kernel = wrapper
    bass2jax._ant_multiwait_hooked = True


# ---- problem constants (hardcoded per harness contract) ----
B, N, C, H = 4, 1024, 1024, 16
HD = C // H            # 64
HID = 4 * C            # 4096
EPS = 1e-5
P = 128
KC = C // P            # 8 contraction chunks over C
HJ = HID // P          # 32 chunks over hidden
TQ = N // 2            # 512 local query tokens per core
VW = 2 * HD            # v tile width: 64 v dims + 64 ones
NCORES = 8
SHUF_SWAP16 = [(i + 16) % 32 for i in range(32)]  # re<->im 16-block swap

F32 = mybir.dt.float32
F32R = mybir.dt.float32r
BF16 = mybir.dt.bfloat16
FT = mybir.ActivationFunctionType
OP = mybir.AluOpType


# ----------------------------------------------------------------------------
# Bass program (identical for every core)
# ----------------------------------------------------------------------------

def build_nc(reps=1):
    nc = bass.Bass("TRN2", target_bir_lowering=False, debug=False)

    # -------- DRAM I/O --------
    d_xT = nc.dram_tensor("xT", [C, N], F32, kind="ExternalInput").ap()
    d_trig = nc.dram_tensor("trig", [P, 2 * N], BF16, kind="ExternalInput").ap()
    d_ones = nc.dram_tensor("onesT", [P, H * HD], BF16, kind="ExternalInput").ap()
    d_cvec = nc.dram_tensor("cvec", [P, 6 * KC + HJ], F32, kind="ExternalInput").ap()
    d_wq = nc.dram_tensor("wq", [KC, P, C], BF16, kind="ExternalInput").ap()
    d_wk = nc.dram_tensor("wk", [KC, P, C], BF16, kind="ExternalInput").ap()
    d_wv = nc.dram_tensor("wv", [P, KC, C], BF16, kind="ExternalInput").ap()
    d_wp = nc.dram_tensor("wp", [KC, P, C], BF16, kind="ExternalInput").ap()
    d_wf1 = nc.dram_tensor("wf1", [HJ, P, C], BF16, kind="ExternalInput").ap()
    d_wf2 = nc.dram_tensor("wf2", [KC, P, HID], BF16, kind="ExternalInput").ap()
    d_out = nc.dram_tensor("outT", [KC, P, TQ], BF16, kind="ExternalOutput").ap()

    xT_t = d_xT.rearrange("(kc p) t -> p kc t", p=P)  # [128, 8, 1024]

    with tile.TileContext(nc) as tc, ExitStack() as top:
        const = top.enter_context(tc.tile_pool(name="const", bufs=1))

        # ---- constants ----
        eps_t = const.tile([P, 1], F32, tag="eps")
        nc.vector.memset(eps_t, EPS)
        ones128 = const.tile([P, P], BF16, tag="ones128")
        nc.sync.dma_start(out=ones128, in_=d_ones[:, 0:P])

        cvec = const.tile([P, 6 * KC + HJ], F32, tag="cvec")
        nc.sync.dma_start(out=cvec, in_=d_cvec)
        ln1g = cvec[:, 0 * KC:1 * KC]
        ln1b = cvec[:, 1 * KC:2 * KC]
        ln2g = cvec[:, 2 * KC:3 * KC]
        ln2b = cvec[:, 3 * KC:4 * KC]
        bp = cvec[:, 4 * KC:5 * KC]
        bf2 = cvec[:, 5 * KC:6 * KC]
        bf1 = cvec[:, 6 * KC:6 * KC + HJ]
        trig = const.tile([P, 2 * N], BF16, tag="trig")
        nc.sync.dma_start(out=trig, in_=d_trig)
        cosR = trig[:, 0:N]
        sinPM = trig[:, N:2 * N]

        def emit(rep):
            big = tc.alloc_tile_pool(name=f"big{rep}", bufs=1)
            # ---- long-lived activations ----
            # t16a slot: xloc fp32 (phases A-D), then h2+rb bf16 (E-G)
            xloc = big.tile([P, KC, TQ], F32, tag="t16a")
            nc.sync.dma_start(out=xloc[:, 0:4, :], in_=xT_t[:, 0:4, 0:TQ])
            nc.sync.dma_start(out=xloc[:, 4:KC, :], in_=xT_t[:, 4:KC, 0:TQ])
            osb = big.tile([P, KC, TQ], BF16, tag="osb")   # attention out (o^T)
            resid = big.tile([P, KC, TQ], F32, tag="resid")  # x + attn

            # feature-major layernorm: mean/rstd replicated on all 128
            # partitions, bf16 streams; rstd = Exp(-0.5*Ln(var+eps)).
            def ln_stats(src_tiles, width, psumpool, wk, m_rep, r_rep):
                for hf in range(width // 512):
                    sl = slice(hf * 512, hf * 512 + 512)
                    ps_s = psumpool.tile([P, 512], F32, tag="ps_stat_s")
                    ps_q = psumpool.tile([P, 512], F32, tag="ps_stat_q")
                    for kc in range(KC):
                        xpart = src_tiles(kc, hf)
                        nc.tensor.matmul(ps_s, lhsT=ones128, rhs=xpart,
                                         start=(kc == 0), stop=(kc == KC - 1))
                        sq = wk.tile([P, 512], BF16, tag="ln_sq")
                        nc.vector.tensor_mul(sq, xpart, xpart)
                        nc.tensor.matmul(ps_q, lhsT=ones128, rhs=sq,
                                         start=(kc == 0), stop=(kc == KC - 1))
                    nc.scalar.mul(m_rep[:, sl], ps_s, 1.0 / C)
                    qrep = wk.tile([P, 512], F32, tag="ln_qrep")
                    nc.scalar.mul(qrep, ps_q, 1.0 / C)
                    # var = E[x^2] - mean^2 (bf16 mean -> widen via mul to f32)
                    vrep = wk.tile([P, 512], F32, tag="ln_vrep")
                    nc.vector.tensor_mul(vrep, m_rep[:, sl], m_rep[:, sl])
                    nc.vector.tensor_sub(vrep, qrep, vrep)
                    # rstd = exp(-0.5 * ln(var + eps))
                    lrep = wk.tile([P, 512], F32, tag="ln_lrep")
                    nc.scalar.activation(lrep, vrep, FT.Ln, bias=eps_t)
                    nc.scalar.activation(r_rep[:, sl], lrep, FT.Exp, scale=-0.5)

            def ln_apply(wk, xpart, m_sl, r_sl, g, b, kc, out_ap, eng=None):
                eng = eng or nc.vector
                t1 = wk.tile([P, 512], BF16, tag="ln_t1")
                eng.tensor_sub(t1, xpart, m_sl)
                eng.tensor_mul(t1, t1, r_sl)
                eng.tensor_scalar(
                    out=out_ap, in0=t1,
                    scalar1=g[:, kc:kc + 1], scalar2=b[:, kc:kc + 1],
                    op0=OP.mult, op1=OP.add)

            # phase-A pool on the right side (non-LIFO release vs attn pool)
            phA_cm = tc.tile_pool(name=f"phA{rep}", bufs=1, side="right")
            pA = phA_cm.__enter__()
            h1 = pA.tile([P, KC, N], BF16, tag="h1")       # LN1 out (16KB/part)

            # attention-span pool (qsb written during phase A by the Q block)
            attn_cm = tc.tile_pool(name=f"attn{rep}", bufs=1)
            pAT = attn_cm.__enter__()
            # vsb[p, tj, head, 0:64] = v[token tj*128+p, head*64+d]
            # vsb[p, tj, head, 64:128] = 1.0  (softmax-denominator trick)
            vsb = pAT.tile([P, KC, H, VW], BF16, tag="vsb")   # 32KB/part
            qsb = pAT.tile([P, KC, TQ], BF16, tag="qsb")
            ksb = pAT.tile([P, KC, N], BF16, tag="ksb")
            for tj in range(KC):
                nc.sync.dma_start(
                    out=vsb[:, tj, :, HD:VW],
                    in_=d_ones.rearrange("p (h w) -> p h w", h=H))

            # rope: cast PSUM->bf16 on ScalarE; muls bf16 on DVE; re<->im swap
            # is a single within-quadrant stream_shuffle.
            def rope(out_ap, ps, sl, wk):
                pb = wk.tile([P, 512], BF16, tag="ropepb")
                nc.scalar.copy(pb, ps)
                tpm = wk.tile([P, 512], BF16, tag="ropes")
                nc.vector.tensor_mul(tpm, pb, sinPM[:, sl])
                tsh = wk.tile([P, 512], BF16, tag="ropesh")
                nc.vector.stream_shuffle(tsh, tpm, SHUF_SWAP16)
                tcos = wk.tile([P, 512], BF16, tag="ropec")
                nc.vector.tensor_mul(tcos, pb, cosR[:, sl])
                nc.vector.tensor_add(out_ap, tcos, tsh)

            # ============ Phase A: LN1 (hf-major) ============================
            with ExitStack() as phA:
                wkA = phA.enter_context(tc.tile_pool(name=f"wkA{rep}", bufs=3))
                psA = phA.enter_context(tc.tile_pool(name=f"psA{rep}", bufs=2, space="PSUM"))
                xrp = phA.enter_context(tc.tile_pool(name=f"xrp{rep}", bufs=1))
                xb = xrp.tile([P, KC, N], BF16, tag="xb")  # bf16 cast of x
                # SWDGE cast-DMA, split so hf=0 stats start as early as possible
                nc.gpsimd.dma_start(out=xb[:, 0:4, 0:TQ], in_=xT_t[:, 0:4, 0:TQ])
                nc.gpsimd.dma_start(out=xb[:, 4:KC, 0:TQ], in_=xT_t[:, 4:KC, 0:TQ])
                nc.gpsimd.dma_start(out=xb[:, :, TQ:N], in_=xT_t[:, :, TQ:N])
                m1 = xrp.tile([P, N], BF16, tag="m1rep")
                r1 = xrp.tile([P, N], BF16, tag="r1rep")

                def src1(kc, hf):
                    return xb[:, kc, hf * 512:hf * 512 + 512]

                # hf-major: local-half stats+apply complete first so the V and
                # Q matmuls (which only need h1 columns 0:512 first) can start
                # at the halfway point of LN1.
                for hf in range(2):
                    sl = slice(hf * 512, hf * 512 + 512)
                    ln_stats(lambda kc, _: src1(kc, hf), 512, psA, wkA,
                             m1[:, sl], r1[:, sl])
                    for kc in range(KC):
                        ln_apply(wkA, src1(kc, hf), m1[:, sl], r1[:, sl],
                                 ln1g, ln1b, kc, h1[:, kc, sl])

            # ================= Phase B1: V = h1 @ wv (token-major) ===========
            # tj-outer so each h1 block is a stationary operand for 2 matmuls.
            with ExitStack() as phB1:
                wvp = phB1.enter_context(tc.tile_pool(name=f"wvp{rep}", bufs=1))
                psB1 = phB1.enter_context(tc.tile_pool(name=f"psB1{rep}", bufs=2, space="PSUM"))
                wvt = wvp.tile([P, KC, C], BF16, tag="wvt")   # full wv, 16KB/part
                nc.sync.dma_start(out=wvt, in_=d_wv)
                for tj in range(KC):
                    ps_v = psB1.tile([P, 2, 512], F32, tag="ps_v")
                    for kc in range(KC):
                        for hf in range(2):
                            nc.tensor.matmul(
                                ps_v[:, hf, :],
                                lhsT=h1[:, kc, tj * P:(tj + 1) * P],
                                rhs=wvt[:, kc, hf * 512:hf * 512 + 512],
                                start=(kc == 0), stop=(kc == KC - 1))
                    for hf in range(2):
                        nc.scalar.copy(
                            vsb[:, tj, hf * KC:(hf + 1) * KC, 0:HD],
                            ps_v[:, hf, :].rearrange("p (h d) -> p h d", h=KC))

            # ================= Phase B2: Q/K + RoPE ==========================
            with ExitStack() as phB2:
                wqp = phB2.enter_context(tc.tile_pool(name=f"wqp{rep}", bufs=2))
                wkB = phB2.enter_context(tc.tile_pool(name=f"wkB{rep}", bufs=3))
                psB2 = phB2.enter_context(tc.tile_pool(name=f"psB2{rep}", bufs=4, space="PSUM"))
                for fj in range(KC):
                    wt = wqp.tile([P, KC, P], BF16, tag="wqkv")
                    nc.sync.dma_start(
                        out=wt, in_=d_wq[fj].rearrange("p (kc f) -> p kc f", kc=KC))
                    ps_q = psB2.tile([P, 512], F32, tag="ps_qk")
                    for kc in range(KC):
                        nc.tensor.matmul(ps_q, lhsT=wt[:, kc, :],
                                         rhs=h1[:, kc, 0:TQ],
                                         start=(kc == 0), stop=(kc == KC - 1))
                    rope(qsb[:, fj, :], ps_q, slice(0, TQ), wkB)
                for fj in range(KC):
                    wt = wqp.tile([P, KC, P], BF16, tag="wqkv")
                    nc.sync.dma_start(
                        out=wt, in_=d_wk[fj].rearrange("p (kc f) -> p kc f", kc=KC))
                    for hf in range(2):
                        sl = slice(hf * 512, hf * 512 + 512)
                        ps_k = psB2.tile([P, 512], F32, tag="ps_qk")
                        for kc in range(KC):
                            nc.tensor.matmul(ps_k, lhsT=wt[:, kc, :],
                                             rhs=h1[:, kc, sl],
                                             start=(kc == 0), stop=(kc == KC - 1))
                        rope(ksb[:, fj, sl], ps_k, sl, wkB)

            phA_cm.__exit__(None, None, None)  # free h1 (16KB/part)

            # proj weight pool on the right side (h1's old space): its DMAs
            # prefetch during attention instead of waiting for the attention
            # pools' SBUF to free up.
            wpp_cm = tc.tile_pool(name=f"wpp{rep}", bufs=3, side="right")
            wpp = wpp_cm.__enter__()

            # ================= Phase C: attention ============================
            with ExitStack() as phC:
                wkC = phC.enter_context(tc.tile_pool(name=f"wkC{rep}", bufs=3))
                psS = phC.enter_context(tc.tile_pool(name=f"psS{rep}", bufs=2, space="PSUM"))
                ps2 = phC.enter_context(tc.tile_pool(name=f"ps2{rep}", bufs=2, space="PSUM"))
                scale = float(HD) ** -0.5
                for j in range(KC):  # head pair j -> heads 2j, 2j+1
                    p2a = ps2.tile([P, TQ], F32, tag="ps2a")
                    p2b = ps2.tile([P, TQ], F32, tag="ps2b")
                    for kc in range(KC):
                        ksl = slice(kc * P, (kc + 1) * P)
                        # both heads' score matmuls: disjoint PE row-groups,
                        # run concurrently into one 2-bank PSUM tile
                        pspair = psS.tile([P, 2, TQ], F32, tag="ps_s2")
                        nc.tensor.matmul(pspair[:, 0, :], lhsT=ksb[0:HD, j, ksl],
                                         rhs=qsb[0:HD, j, :], start=True, stop=True)
                        nc.tensor.matmul(pspair[:, 1, :], lhsT=ksb[HD:P, j, ksl],
                                         rhs=qsb[HD:P, j, :], start=True, stop=True)
                        ea = wkC.tile([P, 2, TQ], BF16, tag="expab")
                        nc.scalar.activation(ea, pspair, FT.Exp, scale=scale)
                        nc.tensor.matmul(p2a, lhsT=vsb[:, kc, 2 * j, :],
                                         rhs=ea[:, 0, :], start=(kc == 0), stop=(kc == KC - 1))
                        nc.tensor.matmul(p2b, lhsT=vsb[:, kc, 2 * j + 1, :],
                                         rhs=ea[:, 1, :], start=(kc == 0), stop=(kc == KC - 1))
                    # softmax normalize (Z replicated 64x at partitions 64:128;
                    # ScalarE copy moves it to 0:64 -- DVE must not read PSUM
                    # for reciprocal and TT ops need a shared start partition)
                    zsa = wkC.tile([HD, TQ], F32, tag="zsa")
                    nc.vector.tensor_copy(zsa, p2a[HD:P, :])
                    rza = wkC.tile([HD, TQ], F32, tag="rza")
                    nc.vector.reciprocal(rza, zsa)
                    nc.vector.tensor_mul(osb[0:HD, j, :], p2a[0:HD, :], rza)
                    zsb = wkC.tile([HD, TQ], F32, tag="zsb")
                    nc.vector.tensor_copy(zsb, p2b[HD:P, :])
                    rzb = wkC.tile([HD, TQ], F32, tag="rzb")
                    nc.vector.reciprocal(rzb, zsb)
                    onb = wkC.tile([HD, TQ], BF16, tag="onb")
                    nc.vector.tensor_mul(onb, p2b[0:HD, :], rzb)
                    nc.gpsimd.tensor_copy(osb[HD:P, j, :], onb)

            attn_cm.__exit__(None, None, None)  # free vsb/qsb/ksb

            # ================= Phase D: proj + residual ======================
            with ExitStack() as phD:
                psD = phD.enter_context(tc.tile_pool(name=f"psD{rep}", bufs=4, space="PSUM"))
                for fj in range(KC):
                    wt = wpp.tile([P, KC, P], BF16, tag="wpt")
                    nc.sync.dma_start(
                        out=wt, in_=d_wp[fj].rearrange("p (kc f) -> p kc f", kc=KC))
                    psp = psD.tile([P, TQ], F32, tag="ps_p")
                    for dj in range(KC):
                        nc.tensor.matmul(psp, lhsT=wt[:, dj, :], rhs=osb[:, dj, :],
                                         start=(dj == 0), stop=(dj == KC - 1))
                    # resid = (psp + b_proj) + x
                    nc.vector.scalar_tensor_tensor(
                        out=resid[:, fj, :], in0=psp, scalar=bp[:, fj:fj + 1],
                        in1=xloc[:, fj, :], op0=OP.add, op1=OP.add)
            wpp_cm.__exit__(None, None, None)

            # h2 + rb share xloc's 16KB slot (t16a) -- xloc dead after phase D
            h2rb = big.tile([P, 2, KC, TQ], BF16, tag="t16a")
            h2 = h2rb[:, 0]
            rb = h2rb[:, 1]                               # bf16 cast of resid

            # ================= Phase E: LN2 ==================================
            with ExitStack() as phE:
                wkE = phE.enter_context(tc.tile_pool(name=f"wkE{rep}", bufs=3))
                psE = phE.enter_context(tc.tile_pool(name=f"psE{rep}", bufs=2, space="PSUM"))
                m2 = wkE.tile([P, TQ], BF16, tag="m2rep")
                r2 = wkE.tile([P, TQ], BF16, tag="r2rep")
                for kc in range(KC):
                    nc.vector.tensor_copy(rb[:, kc, :], resid[:, kc, :])

                def src2(kc, hf):
                    return rb[:, kc, :]

                ln_stats(src2, TQ, psE, wkE, m2, r2)
                for kc in range(KC):
                    ln_apply(wkE, rb[:, kc, :], m2, r2, ln2g, ln2b, kc,
                             h2[:, kc, :])

            # ================= Phase F: fc1 + gelu ===========================
            gsb_cm = tc.tile_pool(name=f"gsbp{rep}", bufs=1)
            pG = gsb_cm.__enter__()
            gsb = pG.tile([P, HJ, TQ], BF16, tag="gsb")        # 32KB/part
            with ExitStack() as phF:
                wf1p = phF.enter_context(tc.tile_pool(name=f"wf1p{rep}", bufs=3))
                psF = phF.enter_context(tc.tile_pool(name=f"psF{rep}", bufs=4, space="PSUM"))
                for hj in range(HJ):
                    wt = wf1p.tile([P, KC, P], BF16, tag="wf1t")
                    nc.sync.dma_start(
                        out=wt, in_=d_wf1[hj].rearrange("p (kc f) -> p kc f", kc=KC))
                    psf = psF.tile([P, TQ], F32, tag="ps_f1")
                    for kc in range(KC):
                        nc.tensor.matmul(psf, lhsT=wt[:, kc, :], rhs=h2[:, kc, :],
                                         start=(kc == 0), stop=(kc == KC - 1))
                    nc.scalar.activation(gsb[:, hj, :], psf, FT.Gelu,
                                         bias=bf1[:, hj:hj + 1])

            # ================= Phase G: fc2 + residual + store ===============
            with ExitStack() as phG:
                wf2p = phG.enter_context(tc.tile_pool(name=f"wf2p{rep}", bufs=2))
                psG = phG.enter_context(tc.tile_pool(name=f"psG{rep}", bufs=4, space="PSUM"))
                wkG = phG.enter_context(tc.tile_pool(name=f"wkG{rep}", bufs=3))
                for fj in range(KC):
                    wt = wf2p.tile([P, HJ, P], BF16, tag="wf2t")
                    nc.sync.dma_start(
                        out=wt, in_=d_wf2[fj].rearrange("p (hj f) -> p hj f", hj=HJ))
                    psf2 = psG.tile([P, TQ], F32, tag="ps_f2")
                    for hj in range(HJ):
                        nc.tensor.matmul(psf2, lhsT=wt[:, hj, :], rhs=gsb[:, hj, :],
                                         start=(hj == 0), stop=(hj == HJ - 1))
                    ot = wkG.tile([P, TQ], BF16, tag="outt")
                    nc.vector.scalar_tensor_tensor(
                        out=ot, in0=psf2, scalar=bf2[:, fj:fj + 1],
                        in1=resid[:, fj, :], op0=OP.add, op1=OP.add)
                    nc.sync.dma_start(out=d_out[fj], in_=ot)
            gsb_cm.__exit__(None, None, None)
            big.release()

        for rep in range(reps):
            emit(rep)

    return nc


# ----------------------------------------------------------------------------
# Host-side input prep
# ----------------------------------------------------------------------------

def _qk_perm():
    """Column permutation for w_q / w_k: feature-tile j holds heads 2j (its
    quadrants 0,1) and 2j+1 (quadrants 2,3); each 32-row quadrant is
    [re(16 pairs) | im(16 pairs)] so the RoPE re<->im swap stays inside a
    32-partition quadrant (one stream_shuffle)."""
    j = np.arange(KC)[:, None, None]
    quad = np.arange(4)[None, :, None]
    r = np.arange(32)[None, None, :]
    head = 2 * j + quad // 2
    pair = 16 * (quad % 2) + (r % 16)
    isim = r // 16
    src = head * HD + 2 * pair + isim
    return src.reshape(-1)


def _rope_rowmap():
    """Row r (0..127) -> RoPE pair index (0..31) for cos/sin row tables."""
    quad = np.arange(4)[:, None]
    r = np.arange(32)[None, :]
    return (16 * (quad % 2) + (r % 16)).reshape(-1)


def _tile_w(w, n_out_tiles):
    """[Cin, Cout] -> [n_out_tiles, 128, (Cin/128)*128]: per out-tile, the
    stationary blocks for every contraction chunk, contiguous."""
    cin = w.shape[0]
    kci = cin // P
    return np.ascontiguousarray(
        w.reshape(kci, P, n_out_tiles, P).transpose(2, 1, 0, 3).reshape(
            n_out_tiles, P, kci * P))


def _col(v, dt=np.float32):
    """[n*128] per-feature vector -> [128, n] per-partition columns."""
    return np.ascontiguousarray(v.reshape(-1, P).T).astype(dt)


_CACHE = {}


def _prep_shared(w_qkv, w_proj, b_proj, w_fc1, b_fc1, w_fc2, b_fc2,
                 ln1_g, ln1_b, ln2_g, ln2_b):
    bf = ml_dtypes.bfloat16
    perm = _qk_perm()
    wq = np.ascontiguousarray(w_qkv[:, 0 * C:1 * C][:, perm])
    wk = np.ascontiguousarray(w_qkv[:, 1 * C:2 * C][:, perm])
    wv = w_qkv[:, 2 * C:3 * C]
    shared = {}
    shared["onesT"] = np.ones((P, H * HD), bf)
    shared["wq"] = _tile_w(wq, KC).astype(bf)
    shared["wk"] = _tile_w(wk, KC).astype(bf)
    # wv is a moving operand -> [p, kc, Cout]
    shared["wv"] = np.ascontiguousarray(wv.reshape(KC, P, C).transpose(1, 0, 2)).astype(bf)
    shared["wp"] = _tile_w(w_proj, KC).astype(bf)
    shared["wf1"] = _tile_w(w_fc1, HJ).astype(bf)
    shared["wf2"] = _tile_w(w_fc2, KC).astype(bf)
    shared["cvec"] = np.concatenate(
        [_col(ln1_g), _col(ln1_b), _col(ln2_g), _col(ln2_b),
         _col(b_proj), _col(b_fc2), _col(b_fc1)], axis=1)
    return shared


def make_in_maps(x, freqs_cos, freqs_sin, shared):
    bf = ml_dtypes.bfloat16
    rowmap = _rope_rowmap()                       # [128] -> pair index
    # sign pattern: +sin on re-rows (0:16 of each quadrant), -sin on im-rows
    sgn = np.tile(np.repeat(np.array([1.0, -1.0], np.float32), 16), 4)[:, None]
    in_maps = []
    for c in range(NCORES):
        b, h = divmod(c, 2)
        order = np.r_[h * TQ:(h + 1) * TQ, (1 - h) * TQ:(2 - h) * TQ]
        xT = np.ascontiguousarray(x[b].T[:, order])
        cosR = np.ascontiguousarray(freqs_cos[b].T[rowmap][:, order]).astype(bf)
        sinPM = np.ascontiguousarray(
            (freqs_sin[b].T[rowmap] * sgn)[:, order]).astype(bf)
        m = {"xT": xT, "trig": np.concatenate([cosR, sinPM], axis=1)}
        m.update(shared)
        in_maps.append(m)
    return in_maps


def prep_all(x, freqs_cos, freqs_sin, ln1_g, ln1_b, w_qkv, w_proj, b_proj,
             ln2_g, ln2_b, w_fc1, b_fc1, w_fc2, b_fc2):
    shared = _prep_shared(
        np.asarray(w_qkv, np.float32), np.asarray(w_proj, np.float32),
        np.asarray(b_proj, np.float32), np.asarray(w_fc1, np.float32),
        np.asarray(b_fc1, np.float32), np.asarray(w_fc2, np.float32),
        np.asarray(b_fc2, np.float32), np.asarray(ln1_g, np.float32),
        np.asarray(ln1_b, np.float32), np.asarray(ln2_g, np.float32),
        np.asarray(ln2_b, np.float32))
    return make_in_maps(np.asarray(x, np.float32),
                        np.asarray(freqs_cos, np.float32),
                        np.asarray(freqs_sin, np.float32), shared)


def gather_out(results):
    out = np.empty((B, N, C), np.float32)
    for c in range(NCORES):
        b, h = divmod(c, 2)
        outT = np.asarray(results[c]["outT"]).astype(np.float32).reshape(C, TQ)
        out[b, h * TQ:(h + 1) * TQ, :] = outT.T
    return out


def kernel(x, freqs_cos, freqs_sin, ln1_g, ln1_b, w_qkv, w_proj, b_proj,
           ln2_g, ln2_b, w_fc1, b_fc1, w_fc2, b_fc2):
    _install_multiwait_hook()
    if "nc" not in _CACHE:
        _CACHE["nc"] = build_nc()
    nc = _CACHE["nc"]
    in_maps = prep_all(x, freqs_cos, freqs_sin, ln1_g, ln1_b, w_qkv, w_proj,
                       b_proj, ln2_g, ln2_b, w_fc1, b_fc1, w_fc2, b_fc2)
    res = run_bass_kernel_spmd(nc, in_maps, core_ids=list(range(NCORES)))
    return gather_out(res.results)



# revision 1
# speedup vs baseline: 1.2034x; 1.2034x over previous
"""Trainium2 Bass kernel for a dense transformer block.

Problem: B=4, N=1024, C=1024, H=16 heads (HD=64), MLP hidden 4096, pre-norm,
RoPE on q/k, exact gelu.

Sharding (8 cores, no collectives): core c handles batch b=c//2 and
sequence-half h=c%2. Each core computes LN1 + K/V over its batch's full 1024
tokens (cheap duplication), and Q / attention / proj / MLP only for its 512
local tokens. Tokens are permuted per-core so the local half is always
columns 0:512 -> all cores run an identical program.

On-chip layout is feature-major (transposed): activations live as [C_part,
token_free]; weights are stationary matmul operands (lhsT), activations
stream as the moving operand. All moving operands are bf16 (fp32(r) streams
at half the PE rate). The host pre-transposes x, pre-tiles all weights into
[out_tile][128, kchunks*128] blocks, and pre-permutes w_q/w_k columns so each
32-partition quadrant holds [re(16)|im(16)] of one head's pair-block.

RoPE: out = in*cosR + qswap(in*sinPM), where sinPM carries the +/- sign per
16-row block and qswap = ONE DVE stream_shuffle (the 16-wide re/im interleave
makes the swap a within-quadrant permute, mask [16..31,0..15]). The PSUM
result is cast to bf16 on ScalarE first; muls are bf16 on DVE.

Attention per head-pair tile j: the two heads' score matmuls (K=64) go to
row-groups {0,1}/{2,3} of the PE array and run concurrently into one
[128,2,512] PSUM pair-tile; one Exp on ScalarE covers both heads; MM2 with
lhsT=[v | ones64] (M=128) yields o_unnorm on partitions 0:64 and the softmax
denominator replicated 64x on 64:128; a DVE copy moves Z to partitions
0:64 (DVE reciprocal must not read PSUM) and vector.reciprocal + one
multiply normalize.

LayerNorm (feature-major): column sums via all-ones [128,128] bf16 stationary
matmuls accumulated over chunks (streams a bf16 cast of the input);
rstd = Exp(-0.5*Ln(var+eps)) on ScalarE -- Ln/Exp share one activation table
set with attention's Exp, so the whole kernel does only two table loads
(natural_log_exp + gelu).

NOTE: toolchain constraints this kernel respects:
- walrus allows only 1 semaphore wait per instruction (excess waits are
  split onto EventSemaphore carriers by a BIR post-pass below)
- accumulating matmuls (start=False) require K=128
- tensor_tensor operands must share the start partition; single-input ops
  (copy/activation) may cross partitions
- scalar.activation Reciprocal/Rsqrt are blocked in bass (accuracy), and
  the custom-DVE reciprocal_approx ops fail walrus codegen in this
  toolchain; LN rstd uses the Ln+Exp trick, softmax uses vector.reciprocal
"""

import json
import ml_dtypes
import numpy as np
from contextlib import ExitStack

import concourse.bass as bass
import concourse.tile as tile
from concourse import mybir
from concourse.bass_utils import run_bass_kernel_spmd

_MAXW = 1


def _split_multiwait(bir_bytes):
    """Move excess per-instruction semaphore waits onto same-engine
    EventSemaphore carriers inserted before the instruction (engine queues
    are in-order, so this is semantically identical)."""
    bir = json.loads(bir_bytes)
    n = [0]
    for fn in bir.get("functions", []):
        for bb in fn.get("blocks", []):
            out = []
            for inst in bb.get("instructions", []):
                si = inst.get("sync_info")
                ow = (si or {}).get("on_wait") or []
                if len(ow) > _MAXW:
                    excess, keep = ow[:-_MAXW], ow[-_MAXW:]
                    for s in range(0, len(excess), _MAXW):
                        n[0] += 1
                        out.append({
                            "debug": inst.get("debug", 0),
                            "engine": inst["engine"],
                            "ins": [],
                            "name": f"antsplitw-{n[0]}",
                            "opcode": "EventSemaphore",
                            "outs": [],
                            "sync_info": {"on_update": [],
                                          "on_wait": excess[s:s + _MAXW]},
                        })
                    si["on_wait"] = keep
                out.append(inst)
            bb["instructions"] = out
    return json.dumps(bir).encode()


def _install_multiwait_hook():
    import concourse.bass2jax as bass2jax
    from concourse import bass_utils as bu
    if getattr(bass2jax, "_ant_multiwait_hooked", False):
        return
    orig = bu.compile_bir_kernel

    def wrapper(bir_json, tmpdir, neff_name="file.neff"):
        if isinstance(bir_json, str):
            bir_json = bir_json.encode()
        return orig(_split_multiwait(bir_json), tmpdir, neff_name)

    bass2jax.compile_bir_kernel = wrapper
    bass2jax._ant_multiwait_hooked = True


# ---- problem constants (hardcoded per harness contract) ----
B, N, C, H = 4, 1024, 1024, 16
HD = C // H            # 64
HID = 4 * C            # 4096
EPS = 1e-5
P = 128
KC = C // P            # 8 contraction chunks over C
HJ = HID // P          # 32 chunks over hidden
TQ = N // 2            # 512 local query tokens per core
VW = 2 * HD            # v tile width: 64 v dims + 64 ones
NCORES = 8
SHUF_SWAP16 = [(i + 16) % 32 for i in range(32)]  # re<->im 16-block swap

F32 = mybir.dt.float32
F32R = mybir.dt.float32r
BF16 = mybir.dt.bfloat16
FT = mybir.ActivationFunctionType
OP = mybir.AluOpType


# ----------------------------------------------------------------------------
# Bass program (identical for every core)
# ----------------------------------------------------------------------------

def build_nc(reps=1):
    nc = bass.Bass("TRN2", target_bir_lowering=False, debug=False)

    # -------- DRAM I/O --------
    d_xT = nc.dram_tensor("xT", [C, N], F32, kind="ExternalInput").ap()
    d_trig = nc.dram_tensor("trig", [P, 2 * N], BF16, kind="ExternalInput").ap()
    d_ones = nc.dram_tensor("onesT", [P, H * HD], BF16, kind="ExternalInput").ap()
    d_cvec = nc.dram_tensor("cvec", [P, 6 * KC + HJ], F32, kind="ExternalInput").ap()
    d_wq = nc.dram_tensor("wq", [KC, P, C], BF16, kind="ExternalInput").ap()
    d_wk = nc.dram_tensor("wk", [KC, P, C], BF16, kind="ExternalInput").ap()
    d_wv = nc.dram_tensor("wv", [P, KC, C], BF16, kind="ExternalInput").ap()
    d_wp = nc.dram_tensor("wp", [KC, P, C], BF16, kind="ExternalInput").ap()
    d_wf1 = nc.dram_tensor("wf1", [HJ, P, C], BF16, kind="ExternalInput").ap()
    d_wf2 = nc.dram_tensor("wf2", [KC, P, HID], BF16, kind="ExternalInput").ap()
    d_out = nc.dram_tensor("outT", [KC, P, TQ], BF16, kind="ExternalOutput").ap()

    xT_t = d_xT.rearrange("(kc p) t -> p kc t", p=P)  # [128, 8, 1024]

    with tile.TileContext(nc) as tc, ExitStack() as top:
        const = top.enter_context(tc.tile_pool(name="const", bufs=1))

        # ---- constants ----
        eps_t = const.tile([P, 1], F32, tag="eps")
        nc.vector.memset(eps_t, EPS)
        ones128 = const.tile([P, P], BF16, tag="ones128")
        nc.sync.dma_start(out=ones128, in_=d_ones[:, 0:P])

        cvec = const.tile([P, 6 * KC + HJ], F32, tag="cvec")
        nc.sync.dma_start(out=cvec, in_=d_cvec)
        ln1g = cvec[:, 0 * KC:1 * KC]
        ln1b = cvec[:, 1 * KC:2 * KC]
        ln2g = cvec[:, 2 * KC:3 * KC]
        ln2b = cvec[:, 3 * KC:4 * KC]
        bp = cvec[:, 4 * KC:5 * KC]
        bf2 = cvec[:, 5 * KC:6 * KC]
        bf1 = cvec[:, 6 * KC:6 * KC + HJ]
        trig = const.tile([P, 2 * N], BF16, tag="trig")
        nc.sync.dma_start(out=trig, in_=d_trig)
        cosR = trig[:, 0:N]
        sinPM = trig[:, N:2 * N]

        def emit(rep):
            big = tc.alloc_tile_pool(name=f"big{rep}", bufs=1)
            # ---- long-lived activations ----
            # t16a slot: xloc fp32 (phases A-D), then h2+rb bf16 (E-G)
            xloc = big.tile([P, KC, TQ], F32, tag="t16a")
            nc.sync.dma_start(out=xloc[:, 0:4, :], in_=xT_t[:, 0:4, 0:TQ])
            nc.sync.dma_start(out=xloc[:, 4:KC, :], in_=xT_t[:, 4:KC, 0:TQ])
            osb = big.tile([P, KC, TQ], BF16, tag="osb")   # attention out (o^T)
            resid = big.tile([P, KC, TQ], F32, tag="resid")  # x + attn

            # feature-major layernorm: mean/rstd replicated on all 128
            # partitions, bf16 streams; rstd = Exp(-0.5*Ln(var+eps)).
            def ln_stats(src_tiles, width, psumpool, wk, m_rep, r_rep):
                for hf in range(width // 512):
                    sl = slice(hf * 512, hf * 512 + 512)
                    ps_s = psumpool.tile([P, 512], F32, tag="ps_stat_s")
                    ps_q = psumpool.tile([P, 512], F32, tag="ps_stat_q")
                    for kc in range(KC):
                        xpart = src_tiles(kc, hf)
                        nc.tensor.matmul(ps_s, lhsT=ones128, rhs=xpart,
                                         start=(kc == 0), stop=(kc == KC - 1))
                        sq = wk.tile([P, 512], BF16, tag="ln_sq")
                        nc.vector.tensor_mul(sq, xpart, xpart)
                        nc.tensor.matmul(ps_q, lhsT=ones128, rhs=sq,
                                         start=(kc == 0), stop=(kc == KC - 1))
                    nc.scalar.mul(m_rep[:, sl], ps_s, 1.0 / C)
                    qrep = wk.tile([P, 512], F32, tag="ln_qrep")
                    nc.scalar.mul(qrep, ps_q, 1.0 / C)
                    # var = E[x^2] - mean^2 (bf16 mean -> widen via mul to f32)
                    vrep = wk.tile([P, 512], F32, tag="ln_vrep")
                    nc.vector.tensor_mul(vrep, m_rep[:, sl], m_rep[:, sl])
                    nc.vector.tensor_sub(vrep, qrep, vrep)
                    # rstd = exp(-0.5 * ln(var + eps))
                    lrep = wk.tile([P, 512], F32, tag="ln_lrep")
                    nc.scalar.activation(lrep, vrep, FT.Ln, bias=eps_t)
                    nc.scalar.activation(r_rep[:, sl], lrep, FT.Exp, scale=-0.5)

            def ln_apply(wk, xpart, m_sl, r_sl, g, b, kc, out_ap, eng=None):
                eng = eng or nc.vector
                t1 = wk.tile([P, 512], BF16, tag="ln_t1")
                eng.tensor_sub(t1, xpart, m_sl)
                eng.tensor_mul(t1, t1, r_sl)
                eng.tensor_scalar(
                    out=out_ap, in0=t1,
                    scalar1=g[:, kc:kc + 1], scalar2=b[:, kc:kc + 1],
                    op0=OP.mult, op1=OP.add)

            # phase-A pool on the right side (non-LIFO release vs attn pool)
            phA_cm = tc.tile_pool(name=f"phA{rep}", bufs=1, side="right")
            pA = phA_cm.__enter__()
            h1 = pA.tile([P, KC, N], BF16, tag="h1")       # LN1 out (16KB/part)

            # attention-span pool (qsb written during phase A by the Q block)
            attn_cm = tc.tile_pool(name=f"attn{rep}", bufs=1)
            pAT = attn_cm.__enter__()
            # vsb[p, tj, head, 0:64] = v[token tj*128+p, head*64+d]
            # vsb[p, tj, head, 64:128] = 1.0  (softmax-denominator trick)
            vsb = pAT.tile([P, KC, H, VW], BF16, tag="vsb")   # 32KB/part
            qsb = pAT.tile([P, KC, TQ], BF16, tag="qsb")
            ksb = pAT.tile([P, KC, N], BF16, tag="ksb")
            for tj in range(KC):
                nc.sync.dma_start(
                    out=vsb[:, tj, :, HD:VW],
                    in_=d_ones.rearrange("p (h w) -> p h w", h=H))

            # rope: cast PSUM->bf16 on ScalarE; muls bf16 on DVE; re<->im swap
            # is a single within-quadrant stream_shuffle.
            def rope(out_ap, ps, sl, wk):
                pb = wk.tile([P, 512], BF16, tag="ropepb")
                nc.scalar.copy(pb, ps)
                tpm = wk.tile([P, 512], BF16, tag="ropes")
                nc.vector.tensor_mul(tpm, pb, sinPM[:, sl])
                tsh = wk.tile([P, 512], BF16, tag="ropesh")
                nc.vector.stream_shuffle(tsh, tpm, SHUF_SWAP16)
                tcos = wk.tile([P, 512], BF16, tag="ropec")
                nc.vector.tensor_mul(tcos, pb, cosR[:, sl])
                nc.vector.tensor_add(out_ap, tcos, tsh)

            # ============ Phase A: LN1 (hf-major) ============================
            with ExitStack() as phA:
                wkA = phA.enter_context(tc.tile_pool(name=f"wkA{rep}", bufs=3))
                psA = phA.enter_context(tc.tile_pool(name=f"psA{rep}", bufs=2, space="PSUM"))
                xrp = phA.enter_context(tc.tile_pool(name=f"xrp{rep}", bufs=1))
                xb = xrp.tile([P, KC, N], BF16, tag="xb")  # bf16 cast of x
                # SWDGE cast-DMA, split so hf=0 stats start as early as possible
                nc.gpsimd.dma_start(out=xb[:, 0:4, 0:TQ], in_=xT_t[:, 0:4, 0:TQ])
                nc.gpsimd.dma_start(out=xb[:, 4:KC, 0:TQ], in_=xT_t[:, 4:KC, 0:TQ])
                nc.gpsimd.dma_start(out=xb[:, :, TQ:N], in_=xT_t[:, :, TQ:N])
                m1 = xrp.tile([P, N], BF16, tag="m1rep")
                r1 = xrp.tile([P, N], BF16, tag="r1rep")

                def src1(kc, hf):
                    return xb[:, kc, hf * 512:hf * 512 + 512]

                # hf-major: local-half stats+apply complete first so the V and
                # Q matmuls (which only need h1 columns 0:512 first) can start
                # at the halfway point of LN1.
                for hf in range(2):
                    sl = slice(hf * 512, hf * 512 + 512)
                    ln_stats(lambda kc, _: src1(kc, hf), 512, psA, wkA,
                             m1[:, sl], r1[:, sl])
                    for kc in range(KC):
                        ln_apply(wkA, src1(kc, hf), m1[:, sl], r1[:, sl],
                                 ln1g, ln1b, kc, h1[:, kc, sl])

            # ================= Phase B1: V = h1 @ wv (token-major) ===========
            # tj-outer so each h1 block is a stationary operand for 2 matmuls.
            with ExitStack() as phB1:
                wvp = phB1.enter_context(tc.tile_pool(name=f"wvp{rep}", bufs=1))
                psB1 = phB1.enter_context(tc.tile_pool(name=f"psB1{rep}", bufs=2, space="PSUM"))
                wvt = wvp.tile([P, KC, C], BF16, tag="wvt")   # full wv, 16KB/part
                nc.sync.dma_start(out=wvt, in_=d_wv)
                for tj in range(KC):
                    ps_v = psB1.tile([P, 2, 512], F32, tag="ps_v")
                    for kc in range(KC):
                        for hf in range(2):
                            nc.tensor.matmul(
                                ps_v[:, hf, :],
                                lhsT=h1[:, kc, tj * P:(tj + 1) * P],
                                rhs=wvt[:, kc, hf * 512:hf * 512 + 512],
                                start=(kc == 0), stop=(kc == KC - 1))
                    for hf in range(2):
                        nc.scalar.copy(
                            vsb[:, tj, hf * KC:(hf + 1) * KC, 0:HD],
                            ps_v[:, hf, :].rearrange("p (h d) -> p h d", h=KC))

            # ================= Phase B2: Q/K + RoPE ==========================
            with ExitStack() as phB2:
                wqp = phB2.enter_context(tc.tile_pool(name=f"wqp{rep}", bufs=2))
                wkB = phB2.enter_context(tc.tile_pool(name=f"wkB{rep}", bufs=3))
                psB2 = phB2.enter_context(tc.tile_pool(name=f"psB2{rep}", bufs=4, space="PSUM"))
                for fj in range(KC):
                    wt = wqp.tile([P, KC, P], BF16, tag="wqkv")
                    nc.sync.dma_start(
                        out=wt, in_=d_wq[fj].rearrange("p (kc f) -> p kc f", kc=KC))
                    ps_q = psB2.tile([P, 512], F32, tag="ps_qk")
                    for kc in range(KC):
                        nc.tensor.matmul(ps_q, lhsT=wt[:, kc, :],
                                         rhs=h1[:, kc, 0:TQ],
                                         start=(kc == 0), stop=(kc == KC - 1))
                    rope(qsb[:, fj, :], ps_q, slice(0, TQ), wkB)
                for fj in range(KC):
                    wt = wqp.tile([P, KC, P], BF16, tag="wqkv")
                    nc.sync.dma_start(
                        out=wt, in_=d_wk[fj].rearrange("p (kc f) -> p kc f", kc=KC))
                    for hf in range(2):
                        sl = slice(hf * 512, hf * 512 + 512)
                        ps_k = psB2.tile([P, 512], F32, tag="ps_qk")
                        for kc in range(KC):
                            nc.tensor.matmul(ps_k, lhsT=wt[:, kc, :],
                                             rhs=h1[:, kc, sl],
                                             start=(kc == 0), stop=(kc == KC - 1))
                        rope(ksb[:, fj, sl], ps_k, sl, wkB)

            phA_cm.__exit__(None, None, None)  # free h1 (16KB/part)

            # proj weight pool on the right side (h1's old space): its DMAs
            # prefetch during attention instead of waiting for the attention
            # pools' SBUF to free up.
            wpp_cm = tc.tile_pool(name=f"wpp{rep}", bufs=3, side="right")
            wpp = wpp_cm.__enter__()

            # ================= Phase C: attention ============================
            with ExitStack() as phC:
                wkC = phC.enter_context(tc.tile_pool(name=f"wkC{rep}", bufs=3))
                psS = phC.enter_context(tc.tile_pool(name=f"psS{rep}", bufs=2, space="PSUM"))
                ps2 = phC.enter_context(tc.tile_pool(name=f"ps2{rep}", bufs=2, space="PSUM"))
                scale = float(HD) ** -0.5
                for j in range(KC):  # head pair j -> heads 2j, 2j+1
                    p2a = ps2.tile([P, TQ], F32, tag="ps2a")
                    p2b = ps2.tile([P, TQ], F32, tag="ps2b")
                    for kc in range(KC):
                        ksl = slice(kc * P, (kc + 1) * P)
                        # both heads' score matmuls: disjoint PE row-groups,
                        # run concurrently into one 2-bank PSUM tile
                        pspair = psS.tile([P, 2, TQ], F32, tag="ps_s2")
                        nc.tensor.matmul(pspair[:, 0, :], lhsT=ksb[0:HD, j, ksl],
                                         rhs=qsb[0:HD, j, :], start=True, stop=True)
                        nc.tensor.matmul(pspair[:, 1, :], lhsT=ksb[HD:P, j, ksl],
                                         rhs=qsb[HD:P, j, :], start=True, stop=True)
                        ea = wkC.tile([P, 2, TQ], BF16, tag="expab")
                        nc.scalar.activation(ea, pspair, FT.Exp, scale=scale)
                        nc.tensor.matmul(p2a, lhsT=vsb[:, kc, 2 * j, :],
                                         rhs=ea[:, 0, :], start=(kc == 0), stop=(kc == KC - 1))
                        nc.tensor.matmul(p2b, lhsT=vsb[:, kc, 2 * j + 1, :],
                                         rhs=ea[:, 1, :], start=(kc == 0), stop=(kc == KC - 1))
                    # softmax normalize (Z replicated 64x at partitions 64:128;
                    # ScalarE copy moves it to 0:64 -- DVE must not read PSUM
                    # for reciprocal and TT ops need a shared start partition)
                    zsa = wkC.tile([HD, TQ], F32, tag="zsa")
                    nc.vector.tensor_copy(zsa, p2a[HD:P, :])
                    rza = wkC.tile([HD, TQ], F32, tag="rza")
                    nc.vector.reciprocal(rza, zsa)
                    nc.vector.tensor_mul(osb[0:HD, j, :], p2a[0:HD, :], rza)
                    zsb = wkC.tile([HD, TQ], F32, tag="zsb")
                    nc.vector.tensor_copy(zsb, p2b[HD:P, :])
                    rzb = wkC.tile([HD, TQ], F32, tag="rzb")
                    nc.vector.reciprocal(rzb, zsb)
                    onb = wkC.tile([HD, TQ], BF16, tag="onb")
                    nc.vector.tensor_mul(onb, p2b[0:HD, :], rzb)
                    nc.gpsimd.tensor_copy(osb[HD:P, j, :], onb)

            attn_cm.__exit__(None, None, None)  # free vsb/qsb/ksb

            # ================= Phase D: proj + residual ======================
            with ExitStack() as phD:
                psD = phD.enter_context(tc.tile_pool(name=f"psD{rep}", bufs=4, space="PSUM"))
                for fj in range(KC):
                    wt = wpp.tile([P, KC, P], BF16, tag="wpt")
                    nc.sync.dma_start(
                        out=wt, in_=d_wp[fj].rearrange("p (kc f) -> p kc f", kc=KC))
                    psp = psD.tile([P, TQ], F32, tag="ps_p")
                    for dj in range(KC):
                        nc.tensor.matmul(psp, lhsT=wt[:, dj, :], rhs=osb[:, dj, :],
                                         start=(dj == 0), stop=(dj == KC - 1))
                    # resid = (psp + b_proj) + x
                    nc.vector.scalar_tensor_tensor(
                        out=resid[:, fj, :], in0=psp, scalar=bp[:, fj:fj + 1],
                        in1=xloc[:, fj, :], op0=OP.add, op1=OP.add)
            wpp_cm.__exit__(None, None, None)

            # h2 + rb share xloc's 16KB slot (t16a) -- xloc dead after phase D
            h2rb = big.tile([P, 2, KC, TQ], BF16, tag="t16a")
            h2 = h2rb[:, 0]
            rb = h2rb[:, 1]                               # bf16 cast of resid

            # ================= Phase E: LN2 ==================================
            with ExitStack() as phE:
                wkE = phE.enter_context(tc.tile_pool(name=f"wkE{rep}", bufs=3))
                psE = phE.enter_context(tc.tile_pool(name=f"psE{rep}", bufs=2, space="PSUM"))
                m2 = wkE.tile([P, TQ], BF16, tag="m2rep")
                r2 = wkE.tile([P, TQ], BF16, tag="r2rep")
                for kc in range(KC):
                    nc.vector.tensor_copy(rb[:, kc, :], resid[:, kc, :])

                def src2(kc, hf):
                    return rb[:, kc, :]

                ln_stats(src2, TQ, psE, wkE, m2, r2)
                for kc in range(KC):
                    ln_apply(wkE, rb[:, kc, :], m2, r2, ln2g, ln2b, kc,
                             h2[:, kc, :])

            # ================= Phase F: fc1 + gelu ===========================
            gsb_cm = tc.tile_pool(name=f"gsbp{rep}", bufs=1)
            pG = gsb_cm.__enter__()
            gsb = pG.tile([P, HJ, TQ], BF16, tag="gsb")        # 32KB/part
            with ExitStack() as phF:
                wf1p = phF.enter_context(tc.tile_pool(name=f"wf1p{rep}", bufs=3))
                psF = phF.enter_context(tc.tile_pool(name=f"psF{rep}", bufs=4, space="PSUM"))
                for hj in range(HJ):
                    wt = wf1p.tile([P, KC, P], BF16, tag="wf1t")
                    nc.sync.dma_start(
                        out=wt, in_=d_wf1[hj].rearrange("p (kc f) -> p kc f", kc=KC))
                    psf = psF.tile([P, TQ], F32, tag="ps_f1")
                    for kc in range(KC):
                        nc.tensor.matmul(psf, lhsT=wt[:, kc, :], rhs=h2[:, kc, :],
                                         start=(kc == 0), stop=(kc == KC - 1))
                    nc.scalar.activation(gsb[:, hj, :], psf, FT.Gelu,
                                         bias=bf1[:, hj:hj + 1])

            # ================= Phase G: fc2 + residual + store ===============
            with ExitStack() as phG:
                wf2p = phG.enter_context(tc.tile_pool(name=f"wf2p{rep}", bufs=2))
                psG = phG.enter_context(tc.tile_pool(name=f"psG{rep}", bufs=4, space="PSUM"))
                wkG = phG.enter_context(tc.tile_pool(name=f"wkG{rep}", bufs=3))
                for fj in range(KC):
                    wt = wf2p.tile([P, HJ, P], BF16, tag="wf2t")
                    nc.sync.dma_start(
                        out=wt, in_=d_wf2[fj].rearrange("p (hj f) -> p hj f", hj=HJ))
                    psf2 = psG.tile([P, TQ], F32, tag="ps_f2")
                    for hj in range(HJ):
                        nc.tensor.matmul(psf2, lhsT=wt[:, hj, :], rhs=gsb[:, hj, :],
                                         start=(hj == 0), stop=(hj == HJ - 1))
                    ot = wkG.tile([P, TQ], BF16, tag="outt")
                    nc.vector.scalar_tensor_tensor(
                        out=ot, in0=psf2, scalar=bf2[:, fj:fj + 1],
                        in1=resid[:, fj, :], op0=OP.add, op1=OP.add)
                    nc.sync.dma_start(out=d_out[fj], in_=ot)
            gsb_cm.__exit__(None, None, None)
            big.release()

        for rep in range(reps):
            emit(rep)

    return nc


# ----------------------------------------------------------------------------
# Host-side input prep
# ----------------------------------------------------------------------------

def _qk_perm():
    """Column permutation for w_q / w_k: feature-tile j holds heads 2j (its
    quadrants 0,1) and 2j+1 (quadrants 2,3); each 32-row quadrant is
    [re(16 pairs) | im(16 pairs)] so the RoPE re<->im swap stays inside a
    32-partition quadrant (one stream_shuffle)."""
    j = np.arange(KC)[:, None, None]
    quad = np.arange(4)[None, :, None]
    r = np.arange(32)[None, None, :]
    head = 2 * j + quad // 2
    pair = 16 * (quad % 2) + (r % 16)
    isim = r // 16
    src = head * HD + 2 * pair + isim
    return src.reshape(-1)


def _rope_rowmap():
    """Row r (0..127) -> RoPE pair index (0..31) for cos/sin row tables."""
    quad = np.arange(4)[:, None]
    r = np.arange(32)[None, :]
    return (16 * (quad % 2) + (r % 16)).reshape(-1)


def _tile_w(w, n_out_tiles):
    """[Cin, Cout] -> [n_out_tiles, 128, (Cin/128)*128]: per out-tile, the
    stationary blocks for every contraction chunk, contiguous."""
    cin = w.shape[0]
    kci = cin // P
    return np.ascontiguousarray(
        w.reshape(kci, P, n_out_tiles, P).transpose(2, 1, 0, 3).reshape(
            n_out_tiles, P, kci * P))


def _col(v, dt=np.float32):
    """[n*128] per-feature vector -> [128, n] per-partition columns."""
    return np.ascontiguousarray(v.reshape(-1, P).T).astype(dt)


_CACHE = {}


def _prep_shared(w_qkv, w_proj, b_proj, w_fc1, b_fc1, w_fc2, b_fc2,
                 ln1_g, ln1_b, ln2_g, ln2_b):
    bf = ml_dtypes.bfloat16
    perm = _qk_perm()
    wq = np.ascontiguousarray(w_qkv[:, 0 * C:1 * C][:, perm])
    wk = np.ascontiguousarray(w_qkv[:, 1 * C:2 * C][:, perm])
    wv = w_qkv[:, 2 * C:3 * C]
    shared = {}
    shared["onesT"] = np.ones((P, H * HD), bf)
    shared["wq"] = _tile_w(wq, KC).astype(bf)
    shared["wk"] = _tile_w(wk, KC).astype(bf)
    # wv is a moving operand -> [p, kc, Cout]
    shared["wv"] = np.ascontiguousarray(wv.reshape(KC, P, C).transpose(1, 0, 2)).astype(bf)
    shared["wp"] = _tile_w(w_proj, KC).astype(bf)
    shared["wf1"] = _tile_w(w_fc1, HJ).astype(bf)
    shared["wf2"] = _tile_w(w_fc2, KC).astype(bf)
    shared["cvec"] = np.concatenate(
        [_col(ln1_g), _col(ln1_b), _col(ln2_g), _col(ln2_b),
         _col(b_proj), _col(b_fc2), _col(b_fc1)], axis=1)
    return shared


def make_in_maps(x, freqs_cos, freqs_sin, shared):
    bf = ml_dtypes.bfloat16
    rowmap = _rope_rowmap()                       # [128] -> pair index
    # sign pattern: +sin on re-rows (0:16 of each quadrant), -sin on im-rows
    sgn = np.tile(np.repeat(np.array([1.0, -1.0], np.float32), 16), 4)[:, None]
    in_maps = []
    for c in range(NCORES):
        b, h = divmod(c, 2)
        order = np.r_[h * TQ:(h + 1) * TQ, (1 - h) * TQ:(2 - h) * TQ]
        xT = np.ascontiguousarray(x[b].T[:, order])
        cosR = np.ascontiguousarray(freqs_cos[b].T[rowmap][:, order]).astype(bf)
        sinPM = np.ascontiguousarray(
            (freqs_sin[b].T[rowmap] * sgn)[:, order]).astype(bf)
        m = {"xT": xT, "trig": np.concatenate([cosR, sinPM], axis=1)}
        m.update(shared)
        in_maps.append(m)
    return in_maps


def prep_all(x, freqs_cos, freqs_sin, ln1_g, ln1_b, w_qkv, w_proj, b_proj,
             ln2_g, ln2_b, w_fc1, b_fc1, w_fc2, b_fc2):
    shared = _prep_shared(
        np.asarray(w_qkv, np.float32), np.asarray(w_proj, np.float32),
        np.asarray(b_proj, np.float32), np.asarray(w_fc1, np.float32),
        np.asarray(b_fc1, np.float32), np.asarray(w_fc2, np.float32),
        np.asarray(b_fc2, np.float32), np.asarray(ln1_g, np.float32),
        np.asarray(ln1_b, np.float32), np.asarray(ln2_g, np.float32),
        np.asarray(ln2_b, np.float32))
    return make_in_maps(np.asarray(x, np.float32),
                        np.asarray(freqs_cos, np.float32),
                        np.asarray(freqs_sin, np.float32), shared)


def gather_out(results):
    out = np.empty((B, N, C), np.float32)
    for c in range(NCORES):
        b, h = divmod(c, 2)
        outT = np.asarray(results[c]["outT"]).astype(np.float32).reshape(C, TQ)
        out[b, h * TQ:(h + 1) * TQ, :] = outT.T
    return out


def kernel(x, freqs_cos, freqs_sin, ln1_g, ln1_b, w_qkv, w_proj, b_proj,
           ln2_g, ln2_b, w_fc1, b_fc1, w_fc2, b_fc2):
    _install_multiwait_hook()
    if "nc" not in _CACHE:
        _CACHE["nc"] = build_nc()
    nc = _CACHE["nc"]
    in_maps = prep_all(x, freqs_cos, freqs_sin, ln1_g, ln1_b, w_qkv, w_proj,
                       b_proj, ln2_g, ln2_b, w_fc1, b_fc1, w_fc2, b_fc2)
    res = run_bass_kernel_spmd(nc, in_maps, core_ids=list(range(NCORES)))
    return gather_out(res.results)

